# revision 44
# baseline (speedup 1.0000x reference)
"""DPFP fast-weight cell kernel for Trainium2 (8 NeuronCores, data-parallel).

Problem (hardcoded shapes):
  x  [32, 1024], W [32, 2048, 2048] (+ projection weights)
  out = (rowsum(W_new) * qp) @ Wo + bo ;  W_new = W + outer(dv, kp)
Sharding: pure data-parallel over the batch axis: 4 batch elements per core.
All projection weights are replicated to every core.
"""

import os
import numpy as np
from contextlib import ExitStack

B_FULL = 32
DIM = 1024
INNER = 2048
P = 128
NCORES = 8
B = B_FULL // NCORES          # 4 batch elements per core
TI = INNER // P               # 16 tiles of 128 rows per batch element
KD = DIM // P                 # 8 contraction chunks for DIM
NB = 512                      # PSUM bank width in f32

_CACHE = {}
LAST_RESULTS = None           # test harness introspection


def _build_nc():
    import concourse.bass as bass
    import concourse.bacc as bacc
    import concourse.tile as tile
    import concourse.mybir as mybir
    from concourse.masks import make_identity

    f32 = mybir.dt.float32
    f32r = mybir.dt.float32r
    AF = mybir.ActivationFunctionType
    OP = mybir.AluOpType

    nc = bacc.Bacc("TRN2", target_bir_lowering=False, debug=False)

    x_d = nc.dram_tensor("x", [B, DIM], f32, kind="ExternalInput").ap()
    W_d = nc.dram_tensor("W", [B, INNER, INNER], f32, kind="ExternalInput").ap()
    Wq_d = nc.dram_tensor("Wq", [DIM, DIM], f32, kind="ExternalInput").ap()
    bq_d = nc.dram_tensor("bq", [1, DIM], f32, kind="ExternalInput").ap()
    Wk_d = nc.dram_tensor("Wk", [DIM, DIM], f32, kind="ExternalInput").ap()
    bk_d = nc.dram_tensor("bk", [1, DIM], f32, kind="ExternalInput").ap()
    Wv_d = nc.dram_tensor("Wv", [DIM, INNER], f32, kind="ExternalInput").ap()
    bv_d = nc.dram_tensor("bv", [1, INNER], f32, kind="ExternalInput").ap()
    Wo_d = nc.dram_tensor("Wo", [INNER, DIM], f32, kind="ExternalInput").ap()
    bo_d = nc.dram_tensor("bo", [1, DIM], f32, kind="ExternalInput").ap()
    Wb_d = nc.dram_tensor("Wb", [DIM, 1], f32, kind="ExternalInput").ap()
    bb_d = nc.dram_tensor("bb", [1, 1], f32, kind="ExternalInput").ap()

    Wn_d = nc.dram_tensor("W_new", [B, INNER, INNER], f32, kind="ExternalOutput").ap()
    out_d = nc.dram_tensor("out", [B, DIM], f32, kind="ExternalOutput").ap()
    kp_scratch = nc.dram_tensor("kp_scratch", [B, INNER], f32).ap()

    with tile.TileContext(nc) as tc, ExitStack() as ctx:
        consts = ctx.enter_context(tc.tile_pool(name="consts", bufs=1))
        smalls = ctx.enter_context(tc.tile_pool(name="smalls", bufs=1))
        scratch8 = ctx.enter_context(tc.tile_pool(name="scratch8", bufs=2))
        scratch4 = ctx.enter_context(tc.tile_pool(name="scratch4", bufs=2))
        wstream = ctx.enter_context(tc.tile_pool(name="wstream", bufs=4))
        wpool = ctx.enter_context(tc.tile_pool(name="wpool", bufs=18))
        accp = ctx.enter_context(tc.tile_pool(name="accp", bufs=1))
        kpbp = ctx.enter_context(tc.tile_pool(name="kpbp", bufs=2))
        ttp = ctx.enter_context(tc.tile_pool(name="ttp", bufs=1))

        pp_small = ctx.enter_context(tc.tile_pool(name="pp_small", bufs=2, space="PSUM"))
        pp_tr = ctx.enter_context(tc.tile_pool(name="pp_tr", bufs=2, space="PSUM"))
        pp_cs = ctx.enter_context(tc.tile_pool(name="pp_cs", bufs=1, space="PSUM"))
        pp_out = ctx.enter_context(tc.tile_pool(name="pp_out", bufs=1, space="PSUM"))

        # ---- constants ----
        identity = consts.tile([B, B], f32)
        make_identity(nc, identity[:])
        ones1b = consts.tile([1, B], f32)       # bias-matmul lhsT (K=1, M=B)
        nc.vector.memset(ones1b[:], 1.0)
        ones1p = consts.tile([1, P], f32)       # broadcast lhsT (K=1, M=P)
        nc.vector.memset(ones1p[:], 1.0)
        onescol = consts.tile([P, 1], f32)      # colsum rhs (K=P, N=1)
        nc.vector.memset(onescol[:], 1.0)

        # ---- x transposed: xT[p, c, b] = x[b, c*128+p] ----
        xT = smalls.tile([P, KD, B], f32)
        x_v = x_d.rearrange("b (c p) -> b c p", p=P).transpose([2, 1, 0])
        for c in range(KD):
            nc.sync.dma_start(xT[:, c, :], x_v[:, c, :])

        # ---- projections: out_sb[b, n] = sum_c x[b, c*128+p] * Wx[..] + bias ----
        def proj(Wsrc, bsrc, out_sb, odim, act=None):
            nchunks = max(1, odim // NB)
            nb = min(NB, odim)
            for n in range(nchunks):
                ps = pp_small.tile([B, nb], f32, tag="ps_proj")
                for c in range(KD):
                    wt = wstream.tile([P, nb], f32, tag="wts")
                    nc.sync.dma_start(wt[:], Wsrc[c * P:(c + 1) * P, n * nb:(n + 1) * nb])
                    nc.tensor.matmul(ps[:], xT[:, c, :], wt[:],
                                     start=(c == 0), stop=False)
                bt = wstream.tile([1, nb], f32, tag="bts", bufs=2)
                nc.sync.dma_start(bt[:], bsrc[:, n * nb:(n + 1) * nb])
                nc.tensor.matmul(ps[:], ones1b[:], bt[:], start=False, stop=True)
                nc.scalar.activation(out_sb[:, n * nb:(n + 1) * nb], ps[:],
                                     act if act is not None else AF.Copy)

        q_sb = scratch4.tile([B, DIM], f32, tag="qk")
        proj(Wq_d, bq_d, q_sb, DIM)
        k_sb = scratch4.tile([B, DIM], f32, tag="qk")
        proj(Wk_d, bk_d, k_sb, DIM)
        beta_sb = smalls.tile([B, 1], f32)
        proj(Wb_d, bb_d, beta_sb, 1, act=AF.Sigmoid)

        # ---- dpfp: p[j] = y[j] * y[j-1 mod],  y = [relu(z), relu(-z)] ----
        def dpfp(z_sb):
            y = scratch8.tile([B, INNER], f32, tag="s8", name="y")
            nc.scalar.activation(y[:, 0:DIM], z_sb[:], AF.Relu)
            nc.scalar.activation(y[:, DIM:INNER], z_sb[:], AF.Relu, scale=-1.0)
            yr = scratch8.tile([B, INNER], f32, tag="s8", name="yr")
            nc.vector.tensor_copy(yr[:, 1:INNER], y[:, 0:INNER - 1])
            nc.vector.tensor_copy(yr[:, 0:1], y[:, INNER - 1:INNER])
            nc.vector.tensor_tensor(y[:], y[:], yr[:], OP.mult)   # in place
            return y

        # ---- transposed small tensors: X_T[p, t, b] = X[b, t*128+p] ----
        def transpose_rows(src_sb, dst):
            for t in range(TI):
                tp = pp_tr.tile([P, B], f32, tag="tr")
                nc.tensor.transpose(tp[:], src_sb[:, t * P:(t + 1) * P], identity[:])
                nc.scalar.copy(dst[:, t * B:(t + 1) * B], tp[:])

        kp_sb = dpfp(k_sb)
        nc.sync.dma_start(kp_scratch[:], kp_sb[:])   # for per-batch broadcast reads
        kpT = smalls.tile([P, TI * B], f32)
        transpose_rows(kp_sb, kpT)

        qp_sb = dpfp(q_sb)
        qpT = smalls.tile([P, TI * B], f32)
        transpose_rows(qp_sb, qpT)

        v_sb = scratch8.tile([B, INNER], f32, tag="s8")
        proj(Wv_d, bv_d, v_sb, INNER)
        vT = smalls.tile([P, TI * B], f32)
        transpose_rows(v_sb, vT)

        kpT3 = kpT[:].rearrange("p (t b) -> p t b", b=B)
        qpT3 = qpT[:].rearrange("p (t b) -> p t b", b=B)
        vT3 = vT[:].rearrange("p (t b) -> p t b", b=B)

        # beta broadcast over partitions: betaB[p, b] = beta[b]
        btp = pp_tr.tile([1, B], f32, tag="btr", bufs=1)
        nc.tensor.transpose(btp[:], beta_sb[:], identity[:])
        b1 = smalls.tile([1, B], f32)
        nc.scalar.copy(b1[:], btp[:])
        bps = pp_tr.tile([P, B], f32, tag="tr")
        nc.tensor.matmul(bps[:], ones1p[:], b1[:], start=True, stop=True)
        betaB = smalls.tile([P, B], f32)
        nc.scalar.copy(betaB[:], bps[:])

        dvT = smalls.tile([P, TI * B], f32)
        rsT = smalls.tile([P, TI * B], f32)
        ovT = smalls.tile([P, TI * B], f32)
        dvT3 = dvT[:].rearrange("p (t b) -> p t b", b=B)
        rsT3 = rsT[:].rearrange("p (t b) -> p t b", b=B)
        ovT3 = ovT[:].rearrange("p (t b) -> p t b", b=B)

        out_ps = [pp_out.tile([B, NB], f32, tag=f"outps{n}", name=f"outps{n}")
                  for n in range(DIM // NB)]
        out_sb = scratch4.tile([B, DIM], f32, tag="qk")

        # ---- main per-batch-element loop ----
        # Loads for batch b+1 are emitted tile-interleaved with batch b's
        # stores so the (in-order) Sync DMA ring builds backlog during the
        # update phase; that backlog drains through the colsum->dv stall at
        # each batch boundary instead of letting the DMA engines go idle.
        wt_cur = []
        for t in range(TI):
            w = wpool.tile([P, INNER], f32, tag="w")
            nc.sync.dma_start(w[:], W_d[0, t * P:(t + 1) * P, :])
            wt_cur.append(w)

        for b in range(B):
            wt_b = wt_cur

            # kp broadcast for this batch element (replicating read from DRAM)
            kpb = kpbp.tile([P, INNER], f32, tag="kpb")
            nc.sync.dma_start(kpb[:], kp_scratch[b:b + 1, :].broadcast_to([P, INNER]))

            # partial column sums: acc[p, j] = sum_t W[b, t*128+p, j]
            acc = accp.tile([P, INNER], f32, tag="acc")
            nc.vector.tensor_tensor(acc[:], wt_b[0][:], wt_b[1][:], OP.add)
            for t in range(2, TI):
                nc.vector.tensor_tensor(acc[:], acc[:], wt_b[t][:], OP.add)

            # finish cross-partition reduce, transposed: cs[p, j] = colsum[j*128+p]
            cs_ps = pp_cs.tile([P, TI], f32, tag="cs")
            for j in range(TI):
                nc.tensor.matmul(cs_ps[:, j:j + 1], acc[:, j * P:(j + 1) * P],
                                 onescol[:], start=True, stop=True)

            # dvT[:, :, b] = betaB[:, b] * (vT - cs * kpT)
            vo_t = ttp.tile([P, TI], f32, tag="t0")
            nc.vector.tensor_tensor(vo_t[:], cs_ps[:], kpT3[:, :, b], OP.mult)
            dvv = ttp.tile([P, TI], f32, tag="t1")
            nc.vector.tensor_tensor(dvv[:], vT3[:, :, b], vo_t[:], OP.subtract)
            nc.vector.tensor_scalar_mul(dvT3[:, :, b], dvv[:], betaB[:, b:b + 1])

            # fused rank-1 update + row-sum of W_new, in place; then store
            last = b == B - 1
            wt_next = []
            for t in range(TI):
                col = t * B + b
                nc.vector.scalar_tensor_tensor(
                    wt_b[t][:], kpb[:], dvT[:, col:col + 1], wt_b[t][:],
                    OP.mult, OP.add, accum_out=rsT[:, col:col + 1])
                nc.sync.dma_start(Wn_d[b, t * P:(t + 1) * P, :], wt_b[t][:])

                if not last:
                    w = wpool.tile([P, INNER], f32, tag="w")
                    nc.sync.dma_start(w[:], W_d[b + 1, t * P:(t + 1) * P, :])
                    wt_next.append(w)
                else:
                    # outvT chunk and output projection, streamed with batch 3
                    nc.vector.tensor_tensor(ovT3[:, t, :], rsT3[:, t, :],
                                            qpT3[:, t, :], OP.mult)
                    for n in range(DIM // NB):
                        wo_t = wstream.tile([P, NB], f32, tag="wts")
                        nc.sync.dma_start(wo_t[:], Wo_d[t * P:(t + 1) * P, n * NB:(n + 1) * NB])
                        nc.tensor.matmul(out_ps[n][:], ovT[:, t * B:(t + 1) * B],
                                         wo_t[:], start=(t == 0), stop=False)
                        if t == TI - 1:
                            bt = wstream.tile([1, NB], f32, tag="bts", bufs=2)
                            nc.sync.dma_start(bt[:], bo_d[:, n * NB:(n + 1) * NB])
                            nc.tensor.matmul(out_ps[n][:], ones1b[:], bt[:],
                                             start=False, stop=True)
                            nc.scalar.copy(out_sb[:, n * NB:(n + 1) * NB], out_ps[n][:])
            wt_cur = wt_next

        nc.sync.dma_start(out_d[:], out_sb[:])

    nc.compile()
    return nc


def _get_nc():
    if "nc" not in _CACHE:
        _CACHE["nc"] = _build_nc()
    return _CACHE["nc"]


def _shard_inputs(inputs):
    f = lambda a: np.ascontiguousarray(np.asarray(a, dtype=np.float32))
    x = f(inputs["x"])
    W = f(inputs["W"])
    shared = {
        "Wq": f(inputs["Wq"]), "bq": f(inputs["bq"]).reshape(1, DIM),
        "Wk": f(inputs["Wk"]), "bk": f(inputs["bk"]).reshape(1, DIM),
        "Wv": f(inputs["Wv"]), "bv": f(inputs["bv"]).reshape(1, INNER),
        "Wo": f(inputs["Wo"]), "bo": f(inputs["bo"]).reshape(1, DIM),
        "Wb": f(inputs["Wb"]), "bb": f(inputs["bb"]).reshape(1, 1),
    }
    in_maps = []
    for c in range(NCORES):
        sl = slice(c * B, (c + 1) * B)
        m = {"x": x[sl], "W": W[sl]}
        m.update(shared)
        in_maps.append(m)
    return in_maps


def kernel(**inputs):
    global LAST_RESULTS
    from concourse.bass_utils import run_bass_kernel_spmd

    nc = _get_nc()
    in_maps = _shard_inputs(inputs)
    trace = os.environ.get("KBENCH_TRACE", "") == "1"
    res = run_bass_kernel_spmd(nc, in_maps, list(range(NCORES)), trace=trace)
    LAST_RESULTS = res
    out = np.concatenate([res.results[c]["out"] for c in range(NCORES)], axis=0)
    W_new = np.concatenate([res.results[c]["W_new"] for c in range(NCORES)], axis=0)
    return out, W_new


# revision 47
# speedup vs baseline: 1.2273x; 1.2273x over previous
"""DPFP fast-weight cell kernel for Trainium2 (8 NeuronCores, data-parallel).

Problem (hardcoded shapes):
  x  [32, 1024], W [32, 2048, 2048] (+ projection weights)
  out = (rowsum(W_new) * qp) @ Wo + bo ;  W_new = W + outer(dv, kp)
Sharding: pure data-parallel over the batch axis: 4 batch elements per core.
All projection weights are replicated to every core.
"""

import os
import numpy as np
from contextlib import ExitStack

B_FULL = 32
DIM = 1024
INNER = 2048
P = 128
NCORES = 8
B = B_FULL // NCORES          # 4 batch elements per core
TI = INNER // P               # 16 tiles of 128 rows per batch element
KD = DIM // P                 # 8 contraction chunks for DIM
NB = 512                      # PSUM bank width in f32

_CACHE = {}
LAST_RESULTS = None           # test harness introspection


def _build_nc():
    import concourse.bass as bass
    import concourse.bacc as bacc
    import concourse.tile as tile
    import concourse.mybir as mybir
    from concourse.masks import make_identity

    f32 = mybir.dt.float32
    f32r = mybir.dt.float32r
    AF = mybir.ActivationFunctionType
    OP = mybir.AluOpType

    nc = bacc.Bacc("TRN2", target_bir_lowering=False, debug=False)

    x_d = nc.dram_tensor("x", [B, DIM], f32, kind="ExternalInput").ap()
    W_d = nc.dram_tensor("W", [B, INNER, INNER], f32, kind="ExternalInput").ap()
    Wq_d = nc.dram_tensor("Wq", [DIM, DIM], f32, kind="ExternalInput").ap()
    bq_d = nc.dram_tensor("bq", [1, DIM], f32, kind="ExternalInput").ap()
    Wk_d = nc.dram_tensor("Wk", [DIM, DIM], f32, kind="ExternalInput").ap()
    bk_d = nc.dram_tensor("bk", [1, DIM], f32, kind="ExternalInput").ap()
    Wv_d = nc.dram_tensor("Wv", [DIM, INNER], f32, kind="ExternalInput").ap()
    bv_d = nc.dram_tensor("bv", [1, INNER], f32, kind="ExternalInput").ap()
    Wo_d = nc.dram_tensor("Wo", [INNER, DIM], f32, kind="ExternalInput").ap()
    bo_d = nc.dram_tensor("bo", [1, DIM], f32, kind="ExternalInput").ap()
    Wb_d = nc.dram_tensor("Wb", [DIM, 1], f32, kind="ExternalInput").ap()
    bb_d = nc.dram_tensor("bb", [1, 1], f32, kind="ExternalInput").ap()

    Wn_d = nc.dram_tensor("W_new", [B, INNER, INNER], f32, kind="ExternalOutput").ap()
    out_d = nc.dram_tensor("out", [B, DIM], f32, kind="ExternalOutput").ap()
    kp_scratch = nc.dram_tensor("kp_scratch", [B, INNER], f32).ap()

    with tile.TileContext(nc) as tc, ExitStack() as ctx:
        consts = ctx.enter_context(tc.tile_pool(name="consts", bufs=1))
        smalls = ctx.enter_context(tc.tile_pool(name="smalls", bufs=1))
        scratch8 = ctx.enter_context(tc.tile_pool(name="scratch8", bufs=2))
        scratch4 = ctx.enter_context(tc.tile_pool(name="scratch4", bufs=2))
        wstream = ctx.enter_context(tc.tile_pool(name="wstream", bufs=4))
        wpool = ctx.enter_context(tc.tile_pool(name="wpool", bufs=18))
        accp = ctx.enter_context(tc.tile_pool(name="accp", bufs=1))
        kpbp = ctx.enter_context(tc.tile_pool(name="kpbp", bufs=2))
        ttp = ctx.enter_context(tc.tile_pool(name="ttp", bufs=1))

        pp_small = ctx.enter_context(tc.tile_pool(name="pp_small", bufs=2, space="PSUM"))
        pp_tr = ctx.enter_context(tc.tile_pool(name="pp_tr", bufs=2, space="PSUM"))
        pp_cs = ctx.enter_context(tc.tile_pool(name="pp_cs", bufs=1, space="PSUM"))
        pp_out = ctx.enter_context(tc.tile_pool(name="pp_out", bufs=1, space="PSUM"))

        # ---- constants ----
        identity = consts.tile([B, B], f32)
        make_identity(nc, identity[:])
        ones1b = consts.tile([1, B], f32)       # bias-matmul lhsT (K=1, M=B)
        nc.vector.memset(ones1b[:], 1.0)
        ones1p = consts.tile([1, P], f32)       # broadcast lhsT (K=1, M=P)
        nc.vector.memset(ones1p[:], 1.0)
        onescol = consts.tile([P, 1], f32)      # colsum rhs (K=P, N=1)
        nc.vector.memset(onescol[:], 1.0)

        # ---- x transposed: xT[p, c, b] = x[b, c*128+p] ----
        xT = smalls.tile([P, KD, B], f32)
        x_v = x_d.rearrange("b (c p) -> b c p", p=P).transpose([2, 1, 0])
        for c in range(KD):
            nc.sync.dma_start(xT[:, c, :], x_v[:, c, :])

        # ---- projections: out_sb[b, n] = sum_c x[b, c*128+p] * Wx[..] + bias ----
        def proj(Wsrc, bsrc, out_sb, odim, act=None):
            nchunks = max(1, odim // NB)
            nb = min(NB, odim)
            for n in range(nchunks):
                ps = pp_small.tile([B, nb], f32, tag="ps_proj")
                for c in range(KD):
                    wt = wstream.tile([P, nb], f32, tag="wts")
                    nc.sync.dma_start(wt[:], Wsrc[c * P:(c + 1) * P, n * nb:(n + 1) * nb])
                    nc.tensor.matmul(ps[:], xT[:, c, :], wt[:],
                                     start=(c == 0), stop=False)
                bt = wstream.tile([1, nb], f32, tag="bts", bufs=2)
                nc.sync.dma_start(bt[:], bsrc[:, n * nb:(n + 1) * nb])
                nc.tensor.matmul(ps[:], ones1b[:], bt[:], start=False, stop=True)
                nc.scalar.activation(out_sb[:, n * nb:(n + 1) * nb], ps[:],
                                     act if act is not None else AF.Copy)

        # ---- dpfp: p[j] = y[j] * y[j-1 mod],  y = [relu(z), relu(-z)] ----
        def dpfp(z_sb):
            y = scratch8.tile([B, INNER], f32, tag="s8", name="y")
            nc.scalar.activation(y[:, 0:DIM], z_sb[:], AF.Relu)
            nc.scalar.activation(y[:, DIM:INNER], z_sb[:], AF.Relu, scale=-1.0)
            yr = scratch8.tile([B, INNER], f32, tag="s8", name="yr")
            nc.vector.tensor_copy(yr[:, 1:INNER], y[:, 0:INNER - 1])
            nc.vector.tensor_copy(yr[:, 0:1], y[:, INNER - 1:INNER])
            nc.vector.tensor_tensor(y[:], y[:], yr[:], OP.mult)   # in place
            return y

        # ---- transposed small tensors: X_T[p, t, b] = X[b, t*128+p] ----
        def transpose_rows(src_sb, dst):
            for t in range(TI):
                tp = pp_tr.tile([P, B], f32, tag="tr")
                nc.tensor.transpose(tp[:], src_sb[:, t * P:(t + 1) * P], identity[:])
                nc.scalar.copy(dst[:, t * B:(t + 1) * B], tp[:])

        # Emission order = Sync-ring DMA issue order = dependency order:
        # 1) W[0] tiles (DMA busy immediately), 2) only what batch 0 needs
        # (Wk->kp, Wv->vT, Wb->beta), 3) batch 0, 4) the q/qp pipeline (only
        # needed by batch 3's out projection; its Wq stream fills the
        # batch0->1 boundary), 5) batches 1-3.
        wt_cur = []
        for t in range(TI):
            w = wpool.tile([P, INNER], f32, tag="w")
            nc.sync.dma_start(w[:], W_d[0, t * P:(t + 1) * P, :])
            wt_cur.append(w)

        k_sb = scratch4.tile([B, DIM], f32, tag="qk")
        proj(Wk_d, bk_d, k_sb, DIM)
        kp_sb = dpfp(k_sb)
        nc.sync.dma_start(kp_scratch[:], kp_sb[:])   # for per-batch broadcast reads
        kpT = smalls.tile([P, TI * B], f32)
        transpose_rows(kp_sb, kpT)

        v_sb = scratch8.tile([B, INNER], f32, tag="s8")
        proj(Wv_d, bv_d, v_sb, INNER)
        vT = smalls.tile([P, TI * B], f32)
        transpose_rows(v_sb, vT)

        beta_sb = smalls.tile([B, 1], f32)
        proj(Wb_d, bb_d, beta_sb, 1, act=AF.Sigmoid)

        # beta broadcast over partitions: betaB[p, b] = beta[b]
        btp = pp_tr.tile([1, B], f32, tag="btr", bufs=1)
        nc.tensor.transpose(btp[:], beta_sb[:], identity[:])
        b1 = smalls.tile([1, B], f32)
        nc.scalar.copy(b1[:], btp[:])
        bps = pp_tr.tile([P, B], f32, tag="tr")
        nc.tensor.matmul(bps[:], ones1p[:], b1[:], start=True, stop=True)
        betaB = smalls.tile([P, B], f32)
        nc.scalar.copy(betaB[:], bps[:])

        kpT3 = kpT[:].rearrange("p (t b) -> p t b", b=B)
        vT3 = vT[:].rearrange("p (t b) -> p t b", b=B)

        dvT = smalls.tile([P, TI * B], f32)
        rsT = smalls.tile([P, TI * B], f32)
        ovT = smalls.tile([P, TI * B], f32)
        dvT3 = dvT[:].rearrange("p (t b) -> p t b", b=B)
        rsT3 = rsT[:].rearrange("p (t b) -> p t b", b=B)
        ovT3 = ovT[:].rearrange("p (t b) -> p t b", b=B)

        out_ps = [pp_out.tile([B, NB], f32, tag=f"outps{n}", name=f"outps{n}")
                  for n in range(DIM // NB)]
        out_sb = scratch4.tile([B, DIM], f32, tag="qk")

        qpT = smalls.tile([P, TI * B], f32)
        qpT3 = qpT[:].rearrange("p (t b) -> p t b", b=B)

        def emit_q_pipeline():
            q_sb = scratch4.tile([B, DIM], f32, tag="qk", name="q_sb")
            proj(Wq_d, bq_d, q_sb, DIM)
            qp_sb = dpfp(q_sb)
            transpose_rows(qp_sb, qpT)

        # ---- main per-batch-element loop ----
        # Loads for batch b+1 are emitted tile-interleaved with batch b's
        # stores so the (in-order) Sync DMA ring builds backlog during the
        # update phase; that backlog drains through the colsum->dv stall at
        # each batch boundary instead of letting the DMA engines go idle.
        for b in range(B):
            wt_b = wt_cur

            # kp broadcast for this batch element (replicating read from DRAM)
            kpb = kpbp.tile([P, INNER], f32, tag="kpb")
            nc.sync.dma_start(kpb[:], kp_scratch[b:b + 1, :].broadcast_to([P, INNER]))

            # partial column sums: acc[p, j] = sum_t W[b, t*128+p, j]
            acc = accp.tile([P, INNER], f32, tag="acc")
            nc.vector.tensor_tensor(acc[:], wt_b[0][:], wt_b[1][:], OP.add)
            for t in range(2, TI):
                nc.vector.tensor_tensor(acc[:], acc[:], wt_b[t][:], OP.add)

            # finish cross-partition reduce, transposed: cs[p, j] = colsum[j*128+p]
            cs_ps = pp_cs.tile([P, TI], f32, tag="cs")
            for j in range(TI):
                nc.tensor.matmul(cs_ps[:, j:j + 1], acc[:, j * P:(j + 1) * P],
                                 onescol[:], start=True, stop=True)

            # dvT[:, :, b] = betaB[:, b] * (vT - cs * kpT)
            vo_t = ttp.tile([P, TI], f32, tag="t0")
            nc.vector.tensor_tensor(vo_t[:], cs_ps[:], kpT3[:, :, b], OP.mult)
            dvv = ttp.tile([P, TI], f32, tag="t1")
            nc.vector.tensor_tensor(dvv[:], vT3[:, :, b], vo_t[:], OP.subtract)
            nc.vector.tensor_scalar_mul(dvT3[:, :, b], dvv[:], betaB[:, b:b + 1])

            # fused rank-1 update + row-sum of W_new, in place; then store
            last = b == B - 1
            wt_next = []
            for t in range(TI):
                col = t * B + b
                nc.vector.scalar_tensor_tensor(
                    wt_b[t][:], kpb[:], dvT[:, col:col + 1], wt_b[t][:],
                    OP.mult, OP.add, accum_out=rsT[:, col:col + 1])
                nc.sync.dma_start(Wn_d[b, t * P:(t + 1) * P, :], wt_b[t][:])

                if not last:
                    w = wpool.tile([P, INNER], f32, tag="w")
                    nc.sync.dma_start(w[:], W_d[b + 1, t * P:(t + 1) * P, :])
                    wt_next.append(w)
                else:
                    # outvT chunk and output projection, streamed with batch 3
                    nc.vector.tensor_tensor(ovT3[:, t, :], rsT3[:, t, :],
                                            qpT3[:, t, :], OP.mult)
                    for n in range(DIM // NB):
                        wo_t = wstream.tile([P, NB], f32, tag="wts")
                        nc.sync.dma_start(wo_t[:], Wo_d[t * P:(t + 1) * P, n * NB:(n + 1) * NB])
                        nc.tensor.matmul(out_ps[n][:], ovT[:, t * B:(t + 1) * B],
                                         wo_t[:], start=(t == 0), stop=False)
                        if t == TI - 1:
                            bt = wstream.tile([1, NB], f32, tag="bts", bufs=2)
                            nc.sync.dma_start(bt[:], bo_d[:, n * NB:(n + 1) * NB])
                            nc.tensor.matmul(out_ps[n][:], ones1b[:], bt[:],
                                             start=False, stop=True)
                            nc.scalar.copy(out_sb[:, n * NB:(n + 1) * NB], out_ps[n][:])
            wt_cur = wt_next
            if b == 0:
                emit_q_pipeline()

        nc.sync.dma_start(out_d[:], out_sb[:])

    nc.compile()
    return nc


def _get_nc():
    if "nc" not in _CACHE:
        _CACHE["nc"] = _build_nc()
    return _CACHE["nc"]


def _shard_inputs(inputs):
    f = lambda a: np.ascontiguousarray(np.asarray(a, dtype=np.float32))
    x = f(inputs["x"])
    W = f(inputs["W"])
    shared = {
        "Wq": f(inputs["Wq"]), "bq": f(inputs["bq"]).reshape(1, DIM),
        "Wk": f(inputs["Wk"]), "bk": f(inputs["bk"]).reshape(1, DIM),
        "Wv": f(inputs["Wv"]), "bv": f(inputs["bv"]).reshape(1, INNER),
        "Wo": f(inputs["Wo"]), "bo": f(inputs["bo"]).reshape(1, DIM),
        "Wb": f(inputs["Wb"]), "bb": f(inputs["bb"]).reshape(1, 1),
    }
    in_maps = []
    for c in range(NCORES):
        sl = slice(c * B, (c + 1) * B)
        m = {"x": x[sl], "W": W[sl]}
        m.update(shared)
        in_maps.append(m)
    return in_maps


def kernel(**inputs):
    global LAST_RESULTS
    from concourse.bass_utils import run_bass_kernel_spmd

    nc = _get_nc()
    in_maps = _shard_inputs(inputs)
    trace = os.environ.get("KBENCH_TRACE", "") == "1"
    res = run_bass_kernel_spmd(nc, in_maps, list(range(NCORES)), trace=trace)
    LAST_RESULTS = res
    out = np.concatenate([res.results[c]["out"] for c in range(NCORES)], axis=0)
    W_new = np.concatenate([res.results[c]["W_new"] for c in range(NCORES)], axis=0)
    return out, W_new
